# revision 5
# baseline (speedup 1.0000x reference)
"""Multi-head attention (B=2, S=2048, H=1024, NH=16) on 8 TRN2 NeuronCores.

Sharding: fully data/tensor parallel, no collectives. Core c = (b, hg) with
b = c // 4 (batch), hg = c % 4 (head group of 4 heads = 256 of the 1024
projection output dims). Each core:
  - gets its batch's query/key/value pre-transposed on host to [H, S]
    (so the contraction dim lands on SBUF partitions with fast DMA),
  - projects qT/kT [256, S] and v [S, 256] with its slice of Wq/Wk/Wv,
  - runs flash-style attention per head entirely on-chip:
      scoresT[k, q] = kT_h.T @ qT_h   (PE, f32r)
      p = exp(scores/8 + mask[k])      (ACT, no max-subtraction needed:
                                        scores are O(1) by construction)
      ctxT_unnorm[65, q] = [v_h | 1].T @ p  (PE; row 64 = softmax denom)
      ctxT = ctxT_unnorm / denom       (PE ones-bcast + DVE mul)
  - writes ctxT [256, S]; host transposes + scatters into [B, S, H].
All matmuls use float32r (4-byte fp32 operands at bf16 PE rate for moving
free dim >= 256).
"""

import functools
import sys

if "/opt/trn_rl_repo" not in sys.path:
    sys.path.insert(0, "/opt/trn_rl_repo")

import numpy as np

B, S, H = 2, 2048, 1024
NH, HD = 16, 64
NCORES = 8
GROUPS = 4                # head groups (cores per batch)
DPG = H // GROUPS         # projection dims per core = 256
HPG = DPG // HD           # heads per core = 4
P = 128                   # SBUF partitions
NHC = H // P              # contraction chunks per projection = 8
QB = 512                  # q block (matmul moving free dim)
NQB = S // QB             # 4
NKC = S // P              # k chunks = 16
VA_W = HD + 1             # v_aug cols per head (64 v dims + ones col)


@functools.lru_cache(maxsize=1)
def _build():
    import concourse.bacc as bacc
    import concourse.mybir as mybir
    import concourse.tile as tile

    F32 = mybir.dt.float32
    F32R = mybir.dt.float32r
    Exp = mybir.ActivationFunctionType.Exp
    MULT = mybir.AluOpType.mult
    ADD = mybir.AluOpType.add

    nc = bacc.Bacc()

    xq_d = nc.declare_dram_parameter("xq", [H, S], F32R, isOutput=False)
    xk_d = nc.declare_dram_parameter("xk", [H, S], F32R, isOutput=False)
    xv_d = nc.declare_dram_parameter("xv", [H, S], F32R, isOutput=False)
    wq_d = nc.declare_dram_parameter("wq", [H, DPG], F32R, isOutput=False)
    wk_d = nc.declare_dram_parameter("wk", [H, DPG], F32R, isOutput=False)
    wv_d = nc.declare_dram_parameter("wv", [H, DPG], F32R, isOutput=False)
    bqk_d = nc.declare_dram_parameter("bqk", [P, 4], F32, isOutput=False)
    bv_d = nc.declare_dram_parameter("bv", [1, DPG], F32R, isOutput=False)
    mk_d = nc.declare_dram_parameter("mk", [P, NKC], F32, isOutput=False)
    out_d = nc.declare_dram_parameter("out", [DPG, S], F32, isOutput=True)

    with tile.TileContext(nc) as tc:
        with (
            tc.tile_pool(name="const", bufs=1) as cpool,
            tc.tile_pool(name="proj_out", bufs=1) as projpool,
            tc.tile_pool(name="xt", bufs=6) as xpool,
            tc.tile_pool(name="pexp", bufs=4) as ppool,
            tc.tile_pool(name="small", bufs=3) as spool,
        ):
            # ---- constants / weights resident in SBUF ----
            wq_sb = cpool.tile([P, NHC * DPG], F32R)
            wk_sb = cpool.tile([P, NHC * DPG], F32R)
            wv_sb = cpool.tile([P, NHC * DPG], F32R)
            for hc in range(NHC):
                sl = slice(hc * DPG, (hc + 1) * DPG)
                rows = slice(hc * P, (hc + 1) * P)
                nc.sync.dma_start(wq_sb[:, sl], wq_d[rows, :])
                nc.sync.dma_start(wk_sb[:, sl], wk_d[rows, :])
                nc.sync.dma_start(wv_sb[:, sl], wv_d[rows, :])
            bqk_sb = cpool.tile([P, 4], F32)          # cols: q0 q1 k0 k1
            nc.sync.dma_start(bqk_sb[:], bqk_d[:])
            bv_sb = cpool.tile([1, DPG], F32R)
            nc.sync.dma_start(bv_sb[:], bv_d[:])
            mk_sb = cpool.tile([P, NKC], F32)
            nc.sync.dma_start(mk_sb[:], mk_d[:])
            ones_f32 = cpool.tile([P, P], F32)
            nc.vector.memset(ones_f32[:], 1.0)
            ones_sb = cpool.tile([1, P], F32R)
            nc.vector.tensor_copy(ones_sb[:], ones_f32[0:1, :])
            ones_r = ones_sb[:]

            # ---- persistent projection outputs ----
            # qT/kT: [256, S] as two partition-halves [128, S] (head pairs)
            qT0 = projpool.tile([P, S], F32R)
            qT1 = projpool.tile([P, S], F32R)
            kT0 = projpool.tile([P, S], F32R)
            kT1 = projpool.tile([P, S], F32R)
            # v_aug: per s-chunk, per head: [v_h (64) | ones (1)]
            va_sb = projpool.tile([P, NKC * HPG * VA_W], F32R)
            for sc in range(NKC):
                for h in range(HPG):
                    oc = sc * HPG * VA_W + h * VA_W + HD
                    nc.vector.tensor_copy(
                        va_sb[:, oc : oc + 1], ones_f32[:, 0:1]
                    )

            # ---- phase 1a: k then q projections ----
            with tc.tile_pool(name="psA", bufs=2, space="PSUM") as psA:
                for x_d, w_sb, bcol, dst0, dst1 in (
                    (xk_d, wk_sb, 2, kT0, kT1),
                    (xq_d, wq_sb, 0, qT0, qT1),
                ):
                    for qb in range(NQB):
                        cols = slice(qb * QB, (qb + 1) * QB)
                        ps0 = psA.tile([P, QB], F32, tag="ps0")
                        ps1 = psA.tile([P, QB], F32, tag="ps1")
                        for hc in range(NHC):
                            xt = xpool.tile([P, QB], F32R, tag="xt")
                            nc.sync.dma_start(
                                xt[:], x_d[hc * P : (hc + 1) * P, cols]
                            )
                            nc.tensor.matmul(
                                ps0[:],
                                w_sb[:, hc * DPG : hc * DPG + P],
                                xt[:],
                                start=(hc == 0),
                                stop=(hc == NHC - 1),
                            )
                            nc.tensor.matmul(
                                ps1[:],
                                w_sb[:, hc * DPG + P : (hc + 1) * DPG],
                                xt[:],
                                start=(hc == 0),
                                stop=(hc == NHC - 1),
                            )
                        # evict with bias add (DVE; biases are typically 0)
                        nc.vector.tensor_scalar(
                            dst0[:, cols], ps0[:],
                            bqk_sb[:, bcol : bcol + 1], None, ADD,
                        )
                        nc.vector.tensor_scalar(
                            dst1[:, cols], ps1[:],
                            bqk_sb[:, bcol + 1 : bcol + 2], None, ADD,
                        )

                # ---- phase 1b: v projection (natural [s, d] layout) ----
                for sc in range(NKC):
                    psv = psA.tile([P, DPG], F32, tag="psv")
                    for hc in range(NHC):
                        xvt = xpool.tile([P, P], F32R, tag="xvt")
                        nc.sync.dma_start(
                            xvt[:],
                            xv_d[hc * P : (hc + 1) * P, sc * P : (sc + 1) * P],
                        )
                        nc.tensor.matmul(
                            psv[:],
                            xvt[:],
                            wv_sb[:, hc * DPG : (hc + 1) * DPG],
                            start=(hc == 0),
                            stop=False,
                        )
                    # bias add via rank-1 matmul (bv broadcast to all rows)
                    nc.tensor.matmul(
                        psv[:], ones_r, bv_sb[:], start=False, stop=True
                    )
                    for h in range(HPG):
                        off = sc * HPG * VA_W + h * VA_W
                        nc.vector.tensor_copy(
                            va_sb[:, off : off + HD],
                            psv[:, h * HD : (h + 1) * HD],
                        )

            # ---- phase 2: attention per (head, q-block) ----
            with (
                tc.tile_pool(name="psS", bufs=3, space="PSUM") as psS,
                tc.tile_pool(name="psC", bufs=2, space="PSUM") as psC,
                tc.tile_pool(name="psB", bufs=2, space="PSUM") as psB,
            ):
                for h in range(HPG):
                    qT_t = qT0 if h < 2 else qT1
                    kT_t = kT0 if h < 2 else kT1
                    rows = slice((h % 2) * HD, (h % 2) * HD + HD)
                    for qb in range(NQB):
                        cols = slice(qb * QB, (qb + 1) * QB)
                        ctx_ps = psC.tile([VA_W, QB], F32, tag="ctx")
                        for kc in range(NKC):
                            s_ps = psS.tile([P, QB], F32, tag="s")
                            nc.tensor.matmul(
                                s_ps[:],
                                kT_t[rows, kc * P : (kc + 1) * P],
                                qT_t[rows, cols],
                                start=True,
                                stop=True,
                            )
                            p_t = ppool.tile([P, QB], F32R, tag="p")
                            nc.scalar.activation(
                                p_t[:], s_ps[:], Exp,
                                bias=mk_sb[:, kc : kc + 1], scale=0.125,
                            )
                            off = kc * HPG * VA_W + h * VA_W
                            nc.tensor.matmul(
                                ctx_ps[:],
                                va_sb[:, off : off + VA_W],
                                p_t[:],
                                start=(kc == 0),
                                stop=(kc == NKC - 1),
                            )
                        rec = spool.tile([1, QB], F32R, tag="rec")
                        with nc.allow_low_precision(
                            reason="float32r out is bit-identical to float32"
                        ):
                            nc.vector.reciprocal(rec[:], ctx_ps[HD : HD + 1, :])
                        bc_ps = psB.tile([HD, QB], F32, tag="bc")
                        nc.tensor.matmul(
                            bc_ps[:], ones_sb[:, :HD], rec[:],
                            start=True, stop=True,
                        )
                        bc_sb = spool.tile([HD, QB], F32, tag="bcs")
                        nc.vector.tensor_copy(bc_sb[:], bc_ps[:])
                        o_t = spool.tile([HD, QB], F32, tag="o")
                        nc.vector.tensor_tensor(
                            o_t[:], ctx_ps[:HD, :], bc_sb[:], MULT
                        )
                        nc.sync.dma_start(
                            out_d[h * HD : (h + 1) * HD, cols], o_t[:]
                        )

    nc.compile()
    return nc


def _in_maps(query, key, value, attention_mask, Wq, bq, Wk, bk, Wv, bv):
    q = np.asarray(query, np.float32)
    k = np.asarray(key, np.float32)
    v = np.asarray(value, np.float32)
    m = np.asarray(attention_mask, np.float32)
    Wq = np.asarray(Wq, np.float32)
    Wk = np.asarray(Wk, np.float32)
    Wv = np.asarray(Wv, np.float32)
    bq = np.asarray(bq, np.float32)
    bk = np.asarray(bk, np.float32)
    bv = np.asarray(bv, np.float32)

    xT = [
        (
            np.ascontiguousarray(q[b].T),
            np.ascontiguousarray(k[b].T),
            np.ascontiguousarray(v[b].T),
        )
        for b in range(B)
    ]
    maps = []
    for c in range(NCORES):
        b, hg = divmod(c, GROUPS)
        hs = hg * DPG
        he = hs + DPG
        bqs, bks = bq[hs:he], bk[hs:he]
        bqk = np.stack(
            [bqs[:P], bqs[P:], bks[:P], bks[P:]], axis=1
        ).astype(np.float32)
        maps.append(
            {
                "xq": xT[b][0],
                "xk": xT[b][1],
                "xv": xT[b][2],
                "wq": np.ascontiguousarray(Wq[hs:he, :].T),
                "wk": np.ascontiguousarray(Wk[hs:he, :].T),
                "wv": np.ascontiguousarray(Wv[hs:he, :].T),
                "bqk": np.ascontiguousarray(bqk),
                "bv": np.ascontiguousarray(bv[hs:he].reshape(1, DPG)),
                "mk": np.ascontiguousarray(m[b, 0, 0].reshape(NKC, P).T),
            }
        )
    return maps


def kernel(query, key, value, attention_mask, Wq, bq, Wk, bk, Wv, bv):
    from concourse.bass_utils import run_bass_kernel_spmd

    nc = _build()
    maps = _in_maps(
        query, key, value, attention_mask, Wq, bq, Wk, bk, Wv, bv
    )
    res = run_bass_kernel_spmd(nc, maps, core_ids=list(range(NCORES)))
    out = np.empty((B, S, H), np.float32)
    for c in range(NCORES):
        b, hg = divmod(c, GROUPS)
        out[b, :, hg * DPG : (hg + 1) * DPG] = res.results[c]["out"].T
    return out


# revision 7
# speedup vs baseline: 1.5274x; 1.5274x over previous
"""Multi-head attention (B=2, S=2048, H=1024, NH=16) on 8 TRN2 NeuronCores.

Sharding: fully data/tensor parallel, no collectives. Core c = (b, hg) with
b = c // 4 (batch), hg = c % 4 (head group of 4 heads = 256 of the 1024
projection output dims). Each core:
  - gets its batch's query/key/value pre-transposed on host to [H, S] bf16
    (contraction dim on SBUF partitions, fast contiguous DMA, half traffic),
  - projects qT/kT [256, S] and v [S, 256] with its slice of Wq/Wk/Wv,
  - runs flash-style attention per head entirely on-chip:
      scoresT[k, q] = kT_h.T @ qT_h   (PE, bf16 in / f32 PSUM out)
      p = exp(scores/8 + mask[k])      (ACT; no max-subtraction needed:
                                        scores are O(1) by construction)
      ctxT_unnorm[65, q] = [v_h | 1].T @ p  (PE; row 64 = softmax denom)
      ctxT = ctxT_unnorm / bcast(denom)     (PE ones-bcast + DVE divide)
  - writes ctxT [256, S] f32; host transposes + scatters into [B, S, H].
Attention inner loops run kc-outer / qb-inner so consecutive matmuls share
the stationary operand (fewer LDWEIGHTS).
"""

import functools
import sys

if "/opt/trn_rl_repo" not in sys.path:
    sys.path.insert(0, "/opt/trn_rl_repo")

import numpy as np

B, S, H = 2, 2048, 1024
NH, HD = 16, 64
NCORES = 8
GROUPS = 4                # head groups (cores per batch)
DPG = H // GROUPS         # projection dims per core = 256
HPG = DPG // HD           # heads per core = 4
P = 128                   # SBUF partitions
NHC = H // P              # contraction chunks per projection = 8
QB = 512                  # q block (matmul moving free dim)
NQB = S // QB             # 4
NKC = S // P              # k chunks = 16
VA_W = HD + 1             # v_aug cols per head (64 v dims + ones col)


@functools.lru_cache(maxsize=1)
def _build():
    import concourse.bacc as bacc
    import concourse.mybir as mybir
    import concourse.tile as tile

    F32 = mybir.dt.float32
    F32R = mybir.dt.float32r
    BF16 = mybir.dt.bfloat16
    Exp = mybir.ActivationFunctionType.Exp
    MULT = mybir.AluOpType.mult
    ADD = mybir.AluOpType.add

    nc = bacc.Bacc()

    xq_d = nc.declare_dram_parameter("xq", [H, S], BF16, isOutput=False)
    xk_d = nc.declare_dram_parameter("xk", [H, S], BF16, isOutput=False)
    xv_d = nc.declare_dram_parameter("xv", [H, S], BF16, isOutput=False)
    wq_d = nc.declare_dram_parameter("wq", [H, DPG], BF16, isOutput=False)
    wk_d = nc.declare_dram_parameter("wk", [H, DPG], BF16, isOutput=False)
    wv_d = nc.declare_dram_parameter("wv", [H, DPG], BF16, isOutput=False)
    bqk_d = nc.declare_dram_parameter("bqk", [P, 4], F32, isOutput=False)
    bv_d = nc.declare_dram_parameter("bv", [1, DPG], BF16, isOutput=False)
    mk_d = nc.declare_dram_parameter("mk", [P, NKC], F32, isOutput=False)
    out_d = nc.declare_dram_parameter("out", [S, DPG], F32, isOutput=True)
    id_d = nc.declare_dram_parameter("ident", [P, P], F32, isOutput=False)

    with tile.TileContext(nc) as tc:
        with (
            tc.tile_pool(name="const", bufs=1) as cpool,
            tc.tile_pool(name="proj_out", bufs=1) as projpool,
            tc.tile_pool(name="xt", bufs=6) as xpool,
            tc.tile_pool(name="pexp", bufs=6) as ppool,
            tc.tile_pool(name="small", bufs=3) as spool,
        ):
            # ---- constants / weights resident in SBUF ----
            bqk_sb = cpool.tile([P, 4], F32)          # cols: q0 q1 k0 k1
            nc.sync.dma_start(bqk_sb[:], bqk_d[:])
            bv_sb = cpool.tile([1, DPG], BF16)
            nc.sync.dma_start(bv_sb[:], bv_d[:])
            mk_sb = cpool.tile([P, NKC], F32)
            nc.sync.dma_start(mk_sb[:], mk_d[:])
            ones_bf = cpool.tile([1, P], BF16)
            nc.vector.memset(ones_bf[:], 1.0)
            ones_f32 = cpool.tile([1, P], F32)
            nc.vector.memset(ones_f32[:], 1.0)
            id_sb = cpool.tile([P, P], F32)
            nc.sync.dma_start(id_sb[:], id_d[:])

            wq_sb = cpool.tile([P, NHC * DPG], BF16)
            wk_sb = cpool.tile([P, NHC * DPG], BF16)
            wv_sb = cpool.tile([P, NHC * DPG], BF16)
            for hc in range(NHC):
                sl = slice(hc * DPG, (hc + 1) * DPG)
                rows = slice(hc * P, (hc + 1) * P)
                nc.sync.dma_start(wk_sb[:, sl], wk_d[rows, :])
            for hc in range(NHC):
                sl = slice(hc * DPG, (hc + 1) * DPG)
                rows = slice(hc * P, (hc + 1) * P)
                nc.sync.dma_start(wq_sb[:, sl], wq_d[rows, :])
                nc.sync.dma_start(wv_sb[:, sl], wv_d[rows, :])

            # ---- persistent projection outputs ----
            # qT/kT: [256, S] as two partition-halves [128, S] (head pairs)
            qT0 = projpool.tile([P, S], BF16)
            qT1 = projpool.tile([P, S], BF16)
            kT0 = projpool.tile([P, S], BF16)
            kT1 = projpool.tile([P, S], BF16)
            # v_aug: per s-chunk, per head: [v_h (64) | ones (1)]
            va_sb = projpool.tile([P, NKC * HPG * VA_W], BF16)
            for sc in range(NKC):
                for h in range(HPG):
                    oc = sc * HPG * VA_W + h * VA_W + HD
                    nc.vector.memset(va_sb[:, oc : oc + 1], 1.0)

            # ---- phase 1a: k then q projections ----
            with tc.tile_pool(name="psA", bufs=2, space="PSUM") as psA:
                for x_d, w_sb, bcol, dst0, dst1 in (
                    (xk_d, wk_sb, 2, kT0, kT1),
                    (xq_d, wq_sb, 0, qT0, qT1),
                ):
                    for qb in range(NQB):
                        cols = slice(qb * QB, (qb + 1) * QB)
                        ps0 = psA.tile([P, QB], F32, tag="ps0")
                        ps1 = psA.tile([P, QB], F32, tag="ps1")
                        for hc in range(NHC):
                            xt = xpool.tile([P, QB], BF16, tag="xt")
                            nc.sync.dma_start(
                                xt[:], x_d[hc * P : (hc + 1) * P, cols]
                            )
                            nc.tensor.matmul(
                                ps0[:],
                                w_sb[:, hc * DPG : hc * DPG + P],
                                xt[:],
                                start=(hc == 0),
                                stop=(hc == NHC - 1),
                            )
                            nc.tensor.matmul(
                                ps1[:],
                                w_sb[:, hc * DPG + P : (hc + 1) * DPG],
                                xt[:],
                                start=(hc == 0),
                                stop=(hc == NHC - 1),
                            )
                        # evict with bias add (DVE; biases are typically 0)
                        nc.vector.tensor_scalar(
                            dst0[:, cols], ps0[:],
                            bqk_sb[:, bcol : bcol + 1], None, ADD,
                        )
                        nc.vector.tensor_scalar(
                            dst1[:, cols], ps1[:],
                            bqk_sb[:, bcol + 1 : bcol + 2], None, ADD,
                        )

                # ---- phase 1b: v projection (natural [s, d] layout) ----
                for sc in range(NKC):
                    psv = psA.tile([P, DPG], F32, tag="psv")
                    for hc in range(NHC):
                        xvt = xpool.tile([P, P], BF16, tag="xvt")
                        nc.sync.dma_start(
                            xvt[:],
                            xv_d[hc * P : (hc + 1) * P, sc * P : (sc + 1) * P],
                        )
                        nc.tensor.matmul(
                            psv[:],
                            xvt[:],
                            wv_sb[:, hc * DPG : (hc + 1) * DPG],
                            start=(hc == 0),
                            stop=False,
                        )
                    # bias add via rank-1 matmul (bv broadcast to all rows)
                    nc.tensor.matmul(
                        psv[:], ones_bf[:], bv_sb[:], start=False, stop=True
                    )
                    for h in range(HPG):
                        off = sc * HPG * VA_W + h * VA_W
                        nc.vector.tensor_copy(
                            va_sb[:, off : off + HD],
                            psv[:, h * HD : (h + 1) * HD],
                        )

            # ---- phase 2: attention, kc-outer over qb-pairs so consecutive
            # matmuls share the stationary operand and exp runs on 2-bank
            # [128, 1024] tiles (amortizes ACT per-instruction overhead) ----
            with (
                tc.tile_pool(name="psS", bufs=2, space="PSUM") as psS,
                tc.tile_pool(name="psC", bufs=2, space="PSUM") as psC,
                tc.tile_pool(name="psT", bufs=2, space="PSUM") as psT,
            ):
                for h in range(HPG):
                    qT_t = qT0 if h < 2 else qT1
                    kT_t = kT0 if h < 2 else kT1
                    rows = slice((h % 2) * HD, (h % 2) * HD + HD)
                    for pair in range(NQB // 2):
                        ctx2 = [
                            psC.tile([VA_W, QB], F32, tag="ctx",
                                     name=f"ctx{h}_{pair}_{i}")
                            for i in range(2)
                        ]
                        for kc in range(NKC):
                            off = kc * HPG * VA_W + h * VA_W
                            s2 = psS.tile([P, 2 * QB], F32, tag="s2")
                            for i in range(2):
                                qb = pair * 2 + i
                                nc.tensor.matmul(
                                    s2[:, i * QB : (i + 1) * QB],
                                    kT_t[rows, kc * P : (kc + 1) * P],
                                    qT_t[rows, qb * QB : (qb + 1) * QB],
                                    start=True,
                                    stop=True,
                                )
                            p2 = ppool.tile([P, 2 * QB], BF16, tag="p")
                            nc.scalar.activation(
                                p2[:], s2[:], Exp,
                                bias=mk_sb[:, kc : kc + 1], scale=0.125,
                            )
                            for i in range(2):
                                nc.tensor.matmul(
                                    ctx2[i][:],
                                    va_sb[:, off : off + VA_W],
                                    p2[:, i * QB : (i + 1) * QB],
                                    start=(kc == 0),
                                    stop=(kc == NKC - 1),
                                )
                        # epilogue: transpose to [q, d], normalize, store
                        for i in range(2):
                            qb = pair * 2 + i
                            cs = spool.tile([VA_W, QB], F32, tag="cs")
                            nc.vector.tensor_copy(cs[:], ctx2[i][:])
                            for t in range(NQB):
                                tr = psT.tile([P, VA_W], F32, tag="tr")
                                nc.tensor.transpose(
                                    tr[:],
                                    cs[:, t * P : (t + 1) * P],
                                    id_sb[:VA_W, :VA_W],
                                )
                                rec = spool.tile([P, 1], F32, tag="rec")
                                nc.vector.reciprocal(
                                    rec[:], tr[:, HD : HD + 1]
                                )
                                o_t = spool.tile([P, HD], F32, tag="o")
                                nc.vector.tensor_scalar(
                                    o_t[:], tr[:, :HD], rec[:], None, MULT,
                                )
                                nc.sync.dma_start(
                                    out_d[
                                        qb * QB + t * P : qb * QB + (t + 1) * P,
                                        h * HD : (h + 1) * HD,
                                    ],
                                    o_t[:],
                                )

    nc.compile()
    return nc


def _in_maps(query, key, value, attention_mask, Wq, bq, Wk, bk, Wv, bv):
    import ml_dtypes

    bf16 = ml_dtypes.bfloat16
    q = np.asarray(query, np.float32)
    k = np.asarray(key, np.float32)
    v = np.asarray(value, np.float32)
    m = np.asarray(attention_mask, np.float32)
    Wq = np.asarray(Wq, np.float32)
    Wk = np.asarray(Wk, np.float32)
    Wv = np.asarray(Wv, np.float32)
    bq = np.asarray(bq, np.float32)
    bk = np.asarray(bk, np.float32)
    bv = np.asarray(bv, np.float32)

    xT = [
        (
            np.ascontiguousarray(q[b].T).astype(bf16),
            np.ascontiguousarray(k[b].T).astype(bf16),
            np.ascontiguousarray(v[b].T).astype(bf16),
        )
        for b in range(B)
    ]
    maps = []
    for c in range(NCORES):
        b, hg = divmod(c, GROUPS)
        hs = hg * DPG
        he = hs + DPG
        bqs, bks = bq[hs:he], bk[hs:he]
        bqk = np.stack(
            [bqs[:P], bqs[P:], bks[:P], bks[P:]], axis=1
        ).astype(np.float32)
        maps.append(
            {
                "xq": xT[b][0],
                "xk": xT[b][1],
                "xv": xT[b][2],
                "wq": np.ascontiguousarray(Wq[hs:he, :].T).astype(bf16),
                "wk": np.ascontiguousarray(Wk[hs:he, :].T).astype(bf16),
                "wv": np.ascontiguousarray(Wv[hs:he, :].T).astype(bf16),
                "bqk": np.ascontiguousarray(bqk),
                "bv": bv[hs:he].reshape(1, DPG).astype(bf16),
                "mk": np.ascontiguousarray(m[b, 0, 0].reshape(NKC, P).T),
                "ident": np.eye(P, dtype=np.float32),
            }
        )
    return maps


def kernel(query, key, value, attention_mask, Wq, bq, Wk, bk, Wv, bv):
    from concourse.bass_utils import run_bass_kernel_spmd

    nc = _build()
    maps = _in_maps(
        query, key, value, attention_mask, Wq, bq, Wk, bk, Wv, bv
    )
    res = run_bass_kernel_spmd(nc, maps, core_ids=list(range(NCORES)))
    out = np.empty((B, S, H), np.float32)
    for c in range(NCORES):
        b, hg = divmod(c, GROUPS)
        out[b, :, hg * DPG : (hg + 1) * DPG] = res.results[c]["out"]
    return out


# revision 8
# speedup vs baseline: 1.6089x; 1.0534x over previous
"""Multi-head attention (B=2, S=2048, H=1024, NH=16) on 8 TRN2 NeuronCores.

Sharding: fully data/tensor parallel, no collectives. Core c = (b, hg) with
b = c // 4 (batch), hg = c % 4 (head group of 4 heads = 256 of the 1024
projection output dims). Each core:
  - gets its batch's query/key/value pre-transposed on host to [H, S] bf16
    (contraction dim on SBUF partitions, fast contiguous DMA, half traffic),
  - projects qT/kT [256, S] and v [S, 256] with its slice of Wq/Wk/Wv,
  - runs flash-style attention per head entirely on-chip:
      scoresT[k, q] = kT_h.T @ qT_h   (PE, bf16 in / f32 PSUM out)
      p = exp(scores/8 + mask[k])      (ACT; no max-subtraction needed:
                                        scores are O(1) by construction)
      ctxT_unnorm[65, q] = [v_h | 1].T @ p  (PE; row 64 = softmax denom)
      ctxT = ctxT_unnorm / bcast(denom)     (PE ones-bcast + DVE divide)
  - writes ctxT [256, S] f32; host transposes + scatters into [B, S, H].
Attention inner loops run kc-outer / qb-inner so consecutive matmuls share
the stationary operand (fewer LDWEIGHTS).
"""

import functools
import sys

if "/opt/trn_rl_repo" not in sys.path:
    sys.path.insert(0, "/opt/trn_rl_repo")

import numpy as np

B, S, H = 2, 2048, 1024
NH, HD = 16, 64
NCORES = 8
GROUPS = 4                # head groups (cores per batch)
DPG = H // GROUPS         # projection dims per core = 256
HPG = DPG // HD           # heads per core = 4
P = 128                   # SBUF partitions
NHC = H // P              # contraction chunks per projection = 8
QB = 512                  # q block (matmul moving free dim)
NQB = S // QB             # 4
NKC = S // P              # k chunks = 16
VA_W = HD + 1             # v_aug cols per head (64 v dims + ones col)


@functools.lru_cache(maxsize=1)
def _build():
    import concourse.bacc as bacc
    import concourse.mybir as mybir
    import concourse.tile as tile

    F32 = mybir.dt.float32
    F32R = mybir.dt.float32r
    BF16 = mybir.dt.bfloat16
    Exp = mybir.ActivationFunctionType.Exp
    MULT = mybir.AluOpType.mult
    ADD = mybir.AluOpType.add

    nc = bacc.Bacc()

    xq_d = nc.declare_dram_parameter("xq", [H, S], BF16, isOutput=False)
    xk_d = nc.declare_dram_parameter("xk", [H, S], BF16, isOutput=False)
    xv_d = nc.declare_dram_parameter("xv", [H, S], BF16, isOutput=False)
    wq_d = nc.declare_dram_parameter("wq", [H, DPG], BF16, isOutput=False)
    wk_d = nc.declare_dram_parameter("wk", [H, DPG], BF16, isOutput=False)
    wv_d = nc.declare_dram_parameter("wv", [H, DPG], BF16, isOutput=False)
    bqk_d = nc.declare_dram_parameter("bqk", [P, 6], F32, isOutput=False)
    mk_d = nc.declare_dram_parameter("mk", [P, NKC], F32, isOutput=False)
    out_d = nc.declare_dram_parameter("out", [S, DPG], F32, isOutput=True)
    id_d = nc.declare_dram_parameter("ident", [P, P], F32, isOutput=False)

    with tile.TileContext(nc) as tc:
        with (
            tc.tile_pool(name="const", bufs=1) as cpool,
            tc.tile_pool(name="proj_out", bufs=1) as projpool,
            tc.tile_pool(name="xt", bufs=6) as xpool,
            tc.tile_pool(name="pexp", bufs=6) as ppool,
            tc.tile_pool(name="small", bufs=3) as spool,
        ):
            # ---- constants / weights resident in SBUF ----
            bqk_sb = cpool.tile([P, 6], F32)    # cols: q0 q1 k0 k1 v0 v1
            nc.sync.dma_start(bqk_sb[:], bqk_d[:])
            mk_sb = cpool.tile([P, NKC], F32)
            nc.sync.dma_start(mk_sb[:], mk_d[:])
            id_sb = cpool.tile([P, P], F32)
            nc.sync.dma_start(id_sb[:], id_d[:])
            id_bf = cpool.tile([P, P], BF16)
            nc.vector.tensor_copy(id_bf[:], id_sb[:])

            wq_sb = cpool.tile([P, NHC * DPG], BF16)
            wk_sb = cpool.tile([P, NHC * DPG], BF16)
            wv_sb = cpool.tile([P, NHC * DPG], BF16)
            for hc in range(NHC):
                sl = slice(hc * DPG, (hc + 1) * DPG)
                rows = slice(hc * P, (hc + 1) * P)
                nc.sync.dma_start(wk_sb[:, sl], wk_d[rows, :])
            for hc in range(NHC):
                sl = slice(hc * DPG, (hc + 1) * DPG)
                rows = slice(hc * P, (hc + 1) * P)
                nc.sync.dma_start(wq_sb[:, sl], wq_d[rows, :])
                nc.sync.dma_start(wv_sb[:, sl], wv_d[rows, :])

            # ---- persistent projection outputs ----
            # qT/kT: [256, S] as two partition-halves [128, S] (head pairs)
            qT0 = projpool.tile([P, S], BF16)
            qT1 = projpool.tile([P, S], BF16)
            kT0 = projpool.tile([P, S], BF16)
            kT1 = projpool.tile([P, S], BF16)
            vT0 = projpool.tile([P, S], BF16)
            vT1 = projpool.tile([P, S], BF16)
            # v_aug: per s-chunk, per head: [v_h (64) | ones (1)]
            va_sb = projpool.tile([P, NKC * HPG * VA_W], BF16)
            for sc in range(NKC):
                for h in range(HPG):
                    oc = sc * HPG * VA_W + h * VA_W + HD
                    nc.vector.memset(va_sb[:, oc : oc + 1], 1.0)

            # ---- phase 1a: k then q projections ----
            with tc.tile_pool(name="psA", bufs=2, space="PSUM") as psA:
                for x_d, w_sb, bcol, dst0, dst1 in (
                    (xk_d, wk_sb, 2, kT0, kT1),
                    (xq_d, wq_sb, 0, qT0, qT1),
                    (xv_d, wv_sb, 4, vT0, vT1),
                ):
                    for qb in range(NQB):
                        cols = slice(qb * QB, (qb + 1) * QB)
                        ps0 = psA.tile([P, QB], F32, tag="ps0")
                        ps1 = psA.tile([P, QB], F32, tag="ps1")
                        for hc in range(NHC):
                            xt = xpool.tile([P, QB], BF16, tag="xt")
                            nc.sync.dma_start(
                                xt[:], x_d[hc * P : (hc + 1) * P, cols]
                            )
                            nc.tensor.matmul(
                                ps0[:],
                                w_sb[:, hc * DPG : hc * DPG + P],
                                xt[:],
                                start=(hc == 0),
                                stop=(hc == NHC - 1),
                            )
                            nc.tensor.matmul(
                                ps1[:],
                                w_sb[:, hc * DPG + P : (hc + 1) * DPG],
                                xt[:],
                                start=(hc == 0),
                                stop=(hc == NHC - 1),
                            )
                        # evict with bias add (DVE; biases are typically 0)
                        nc.vector.tensor_scalar(
                            dst0[:, cols], ps0[:],
                            bqk_sb[:, bcol : bcol + 1], None, ADD,
                        )
                        nc.vector.tensor_scalar(
                            dst1[:, cols], ps1[:],
                            bqk_sb[:, bcol + 1 : bcol + 2], None, ADD,
                        )

                # ---- phase 1b: transpose vT -> v natural, build v_aug ----
                for half, src_t in ((0, vT0), (1, vT1)):
                    for sc in range(NKC):
                        vtr = psA.tile([P, P], BF16, tag="vtr")
                        nc.tensor.transpose(
                            vtr[:], src_t[:, sc * P : (sc + 1) * P], id_bf[:]
                        )
                        for j in range(2):
                            h = half * 2 + j
                            off = sc * HPG * VA_W + h * VA_W
                            nc.vector.tensor_copy(
                                va_sb[:, off : off + HD],
                                vtr[:, j * HD : (j + 1) * HD],
                            )

            # ---- phase 2: attention, kc-outer over qb-pairs so consecutive
            # matmuls share the stationary operand and exp runs on 2-bank
            # [128, 1024] tiles (amortizes ACT per-instruction overhead) ----
            with (
                tc.tile_pool(name="psS", bufs=2, space="PSUM") as psS,
                tc.tile_pool(name="psC", bufs=2, space="PSUM") as psC,
                tc.tile_pool(name="psT", bufs=2, space="PSUM") as psT,
            ):
                for h in range(HPG):
                    qT_t = qT0 if h < 2 else qT1
                    kT_t = kT0 if h < 2 else kT1
                    rows = slice((h % 2) * HD, (h % 2) * HD + HD)
                    for pair in range(NQB // 2):
                        ctx2 = [
                            psC.tile([VA_W, QB], F32, tag="ctx",
                                     name=f"ctx{h}_{pair}_{i}")
                            for i in range(2)
                        ]
                        for kc in range(NKC):
                            off = kc * HPG * VA_W + h * VA_W
                            s2 = psS.tile([P, 2 * QB], F32, tag="s2")
                            for i in range(2):
                                qb = pair * 2 + i
                                nc.tensor.matmul(
                                    s2[:, i * QB : (i + 1) * QB],
                                    kT_t[rows, kc * P : (kc + 1) * P],
                                    qT_t[rows, qb * QB : (qb + 1) * QB],
                                    start=True,
                                    stop=True,
                                )
                            p2 = ppool.tile([P, 2 * QB], BF16, tag="p")
                            nc.scalar.activation(
                                p2[:], s2[:], Exp,
                                bias=mk_sb[:, kc : kc + 1], scale=0.125,
                            )
                            for i in range(2):
                                nc.tensor.matmul(
                                    ctx2[i][:],
                                    va_sb[:, off : off + VA_W],
                                    p2[:, i * QB : (i + 1) * QB],
                                    start=(kc == 0),
                                    stop=(kc == NKC - 1),
                                )
                        # epilogue: transpose to [q, d], normalize, store
                        for i in range(2):
                            qb = pair * 2 + i
                            cs = spool.tile([VA_W, QB], F32, tag="cs")
                            nc.vector.tensor_copy(cs[:], ctx2[i][:])
                            for t in range(NQB):
                                tr = psT.tile([P, VA_W], F32, tag="tr")
                                nc.tensor.transpose(
                                    tr[:],
                                    cs[:, t * P : (t + 1) * P],
                                    id_sb[:VA_W, :VA_W],
                                )
                                rec = spool.tile([P, 1], F32, tag="rec")
                                nc.vector.reciprocal(
                                    rec[:], tr[:, HD : HD + 1]
                                )
                                o_t = spool.tile([P, HD], F32, tag="o")
                                nc.vector.tensor_scalar(
                                    o_t[:], tr[:, :HD], rec[:], None, MULT,
                                )
                                nc.sync.dma_start(
                                    out_d[
                                        qb * QB + t * P : qb * QB + (t + 1) * P,
                                        h * HD : (h + 1) * HD,
                                    ],
                                    o_t[:],
                                )

    nc.compile()
    return nc


def _in_maps(query, key, value, attention_mask, Wq, bq, Wk, bk, Wv, bv):
    import ml_dtypes

    bf16 = ml_dtypes.bfloat16
    q = np.asarray(query, np.float32)
    k = np.asarray(key, np.float32)
    v = np.asarray(value, np.float32)
    m = np.asarray(attention_mask, np.float32)
    Wq = np.asarray(Wq, np.float32)
    Wk = np.asarray(Wk, np.float32)
    Wv = np.asarray(Wv, np.float32)
    bq = np.asarray(bq, np.float32)
    bk = np.asarray(bk, np.float32)
    bv = np.asarray(bv, np.float32)

    xT = [
        (
            np.ascontiguousarray(q[b].T).astype(bf16),
            np.ascontiguousarray(k[b].T).astype(bf16),
            np.ascontiguousarray(v[b].T).astype(bf16),
        )
        for b in range(B)
    ]
    maps = []
    for c in range(NCORES):
        b, hg = divmod(c, GROUPS)
        hs = hg * DPG
        he = hs + DPG
        bqs, bks, bvs = bq[hs:he], bk[hs:he], bv[hs:he]
        bqk = np.stack(
            [bqs[:P], bqs[P:], bks[:P], bks[P:], bvs[:P], bvs[P:]], axis=1
        ).astype(np.float32)
        maps.append(
            {
                "xq": xT[b][0],
                "xk": xT[b][1],
                "xv": xT[b][2],
                "wq": np.ascontiguousarray(Wq[hs:he, :].T).astype(bf16),
                "wk": np.ascontiguousarray(Wk[hs:he, :].T).astype(bf16),
                "wv": np.ascontiguousarray(Wv[hs:he, :].T).astype(bf16),
                "bqk": np.ascontiguousarray(bqk),
                "mk": np.ascontiguousarray(m[b, 0, 0].reshape(NKC, P).T),
                "ident": np.eye(P, dtype=np.float32),
            }
        )
    return maps


def kernel(query, key, value, attention_mask, Wq, bq, Wk, bk, Wv, bv):
    from concourse.bass_utils import run_bass_kernel_spmd

    nc = _build()
    maps = _in_maps(
        query, key, value, attention_mask, Wq, bq, Wk, bk, Wv, bv
    )
    res = run_bass_kernel_spmd(nc, maps, core_ids=list(range(NCORES)))
    out = np.empty((B, S, H), np.float32)
    for c in range(NCORES):
        b, hg = divmod(c, GROUPS)
        out[b, :, hg * DPG : (hg + 1) * DPG] = res.results[c]["out"]
    return out
